# revision 4
# baseline (speedup 1.0000x reference)
"""Trainium2 Bass kernel: CorrelatorK3.

Math (per batch b):
    q0 = rbf_0 @ Q0_w.T + Q0_b          [N, N, F]
    q  = rbf_d @ Q_w.T  + Q_b
    r0 = rbf_0 @ R0_w.T + R0_b
    r  = rbf_d @ R_w.T  + R_b
    C[n, j] = sum_{i, f} (q0*q)[n, i, f] * (r0*r)[i, j, f] * 0.02

Sharding: data-parallel over batch B=8 across the 8 NeuronCores (one batch
per core); output is a pure concat. INTERVAL=0.02 is folded into the Q0
weights/bias on the host; inputs are pre-rounded to bf16 on the host so the
on-chip bf16 truncation is lossless.

Per-core pipeline:
  Phase 1 (stream x = flattened (u, v) index over 256*256 rows):
    - DMA natural [x, d] fp32 tiles (rbf0 on sync HWDGE, rbfd on gpsimd
      SWDGE). PE-transposes operate on the bf16 upper halves of the fp32
      words (bitcast, stride 2) with a bf16 identity -> bf16 PSUM out at
      1 cyc/col; ScalarE evacuates psum->SBUF bf16 (2x mode).
    - Projections as bf16 matmuls. Stationary rows 0-63 serve the v<128
      column half (g0) with [Q|R]-stacked weight columns; rows 64-127
      serve v>=128 (g1) with the stacking SWAPPED to [R|Q]; the two
      row-groups run concurrently via tile_position.
    - Bias + products fused with PSUM evacuation (ScalarE + VectorE,
      split for engine balance), fp16 into resident R (x-layout):
        v<128:  R[0:64]  = A^T[f,(n,i)]   R[64:128] = B^T[f,(i,j)]
        v>=128: R[0:64]  = B^T[f,(i,j)]   R[64:128] = A^T[f,(n,i)]
      where A=(0.02(q0+b))(q+b) at x=(n,i), B=(r0+b)(r+b) at x=(i,j).
    - ST staging (gpsimd queue, overlapped per u-block): partition-shift
      the two B-quadrants whose half doesn't match their phase-2 stream:
        ST[0:64,  i*128+j] = B[i<128,  j<128]   (from R rows 64-127)
        ST[64:128,(i-128)*128+(j-128)] = B[i>=128, j>=128] (from rows 0-63)
  Phase 2 (contraction as C^T, two PE row-tile streams):
    C^T[j, n] = sum_i B_i^T[f, j]^T A_i^T[f, n]
    - stream L (rows 0-63):  i<128, stationary = contiguous B_i slices
      (staged ST for j<128, native R for j>=128), moving = strided
      A_i^T[f, :] slice; accumulate pcL0/pcL1.
    - stream H (rows 64-127): i>=128 symmetric; pcH0/pcH1.
    - C^T = pcL + pcH (ScalarE copy + VectorE add), DMA out; the host
      transposes back to C.
"""

import os
import sys

if "/opt/trn_rl_repo" not in sys.path:
    sys.path.insert(0, "/opt/trn_rl_repo")

from contextlib import ExitStack

import ml_dtypes
import numpy as np

import concourse.mybir as mybir
import concourse.tile as tile
from concourse import bacc
from concourse.bass_utils import run_bass_kernel_spmd
from concourse.masks import make_identity

B, N, D, F = 8, 256, 64, 64
X = N * N  # 65536 flattened rows per batch
INTERVAL = 0.02

F32 = mybir.dt.float32
F16 = mybir.dt.float16
BF16 = mybir.dt.bfloat16

UPB = int(os.environ.get("KERNEL_UPB", "4"))  # u-rows per phase-1 block
UB = X // (UPB * N)  # 64 phase-1 blocks
_PHASES = os.environ.get("KERNEL_PHASES", "12")  # debug: "1" or "2" only
_P2SEQ = os.environ.get("KERNEL_P2SEQ", "0") == "1"  # sequential streams
SPL = 384  # s0-g1 column split: [0:SPL] on ScalarE, [SPL:] on VectorE


def _body(ctx, tc, rbf0, rbfd, wpack, bpack, cout):
    nc = tc.nc

    const = ctx.enter_context(tc.tile_pool(name="const", bufs=1))
    w_sb = const.tile([128, 256], BF16)
    b_sb = const.tile([128, 4], F32)
    ident = const.tile([128, 128], BF16)
    nc.sync.dma_start(w_sb[:], wpack[:])
    nc.sync.dma_start(b_sb[:], bpack[:])
    make_identity(nc, ident[:])

    res_pool = ctx.enter_context(tc.tile_pool(name="res", bufs=1))
    R = res_pool.tile([128, X], F16)
    R3 = R[:].rearrange("p (u v) -> p u v", v=N)  # [128, 256, 256]
    ST = res_pool.tile([128, 128 * 128], F16)  # staged B quadrants

    rbf0v = rbf0[:].rearrange("(c t p) d -> c p t d", t=2 * UPB, p=128)
    rbfdv = rbfd[:].rearrange("(c t p) d -> c p t d", t=2 * UPB, p=128)

    if "1" in _PHASES:
        _phase1(tc, rbf0v, rbfdv, w_sb, b_sb, ident, R3, ST)
    if "2" in _PHASES:
        _phase2(tc, R3, ST, cout)


def _phase1(tc, rbf0v, rbfdv, w_sb, b_sb, ident, R3, ST):
    nc = tc.nc
    Copy = mybir.ActivationFunctionType.Copy
    Ident = mybir.ActivationFunctionType.Identity
    Alu = mybir.AluOpType
    HB = UPB * 128  # half-block columns (one row-group's share)
    b0g0, b0g1 = b_sb[:, 0:1], b_sb[:, 1:2]
    bdg0, bdg1 = b_sb[:, 2:3], b_sb[:, 3:4]
    with (
        tc.tile_pool(name="chunk", bufs=3) as chunk_pool,
        tc.tile_pool(name="rbfT", bufs=2) as rbfT_pool,
        tc.tile_pool(name="s0p", bufs=2) as s0_pool,
        tc.tile_pool(name="pt", bufs=2, space="PSUM") as pt_pool,
        tc.tile_pool(
            name="pp", bufs=(2 if UPB <= 2 else 1), space="PSUM"
        ) as pp_pool,
    ):
        for ub in range(UB):
            ch0 = chunk_pool.tile([128, HB], F32, tag="ch0")
            chd = chunk_pool.tile([128, HB], F32, tag="chd")
            nc.sync.dma_start(
                ch0[:].rearrange("p (t d) -> p t d", d=D), rbf0v[ub]
            )
            nc.gpsimd.dma_start(
                chd[:].rearrange("p (t d) -> p t d", d=D), rbfdv[ub]
            )

            # bf16 view of the fp32 words: [p, col, lo/hi]; hi = bf16 trunc
            c0h = ch0[:].bitcast(BF16).rearrange("p (c two) -> p c two", two=2)
            cdh = chd[:].bitcast(BF16).rearrange("p (c two) -> p c two", two=2)

            # transpose UPB [128, 128] bf16 sub-blocks per side into one
            # psum tile (each transpose stays inside one bank)
            pt = pt_pool.tile([128, 2 * HB], BF16, tag="pt")
            for j in range(UPB):
                sl = slice(128 * j, 128 * (j + 1))
                sld = slice(HB + 128 * j, HB + 128 * (j + 1))
                nc.tensor.transpose(pt[:, sl], c0h[:, sl, 1], ident[:])
                nc.tensor.transpose(pt[:, sld], cdh[:, sl, 1], ident[:])

            # evacuate both sides' transposes (bf16 2x mode on ScalarE)
            tt = rbfT_pool.tile([128, 2 * HB], BF16, tag="tt")
            nc.scalar.activation(tt[:], pt[:], Copy)

            # projections: wide psum per side, col-halves per row-group.
            # g0 (v<128) stationary rows 0-63: [Q|R]; g1 rows 64-127: [R|Q]
            pp0 = pp_pool.tile([128, 2 * HB], F32, tag="pp0")
            ppd = pp_pool.tile([128, 2 * HB], F32, tag="ppd")
            nc.tensor.matmul(
                pp0[:, 0:HB], w_sb[0:64, 0:128], tt[0:64, 0:HB],
                start=True, stop=True, tile_position=(0, 0),
            )
            nc.tensor.matmul(
                ppd[:, 0:HB], w_sb[0:64, 128:256], tt[0:64, HB : 2 * HB],
                start=True, stop=True, tile_position=(0, 0),
            )
            nc.tensor.matmul(
                pp0[:, HB : 2 * HB], w_sb[64:128, 0:128], tt[64:128, 0:HB],
                start=True, stop=True, tile_position=(64, 0),
            )
            nc.tensor.matmul(
                ppd[:, HB : 2 * HB], w_sb[64:128, 128:256],
                tt[64:128, HB : 2 * HB],
                start=True, stop=True, tile_position=(64, 0),
            )

            # side-0 bias evacuation, split for engine balance
            s0 = s0_pool.tile([128, 2 * HB], F32, tag="s0")
            nc.scalar.activation(
                s0[:, 0:HB], pp0[:, 0:HB], Ident, bias=b0g0
            )
            nc.scalar.activation(
                s0[:, HB : HB + SPL], pp0[:, HB : HB + SPL], Ident, bias=b0g1
            )
            nc.vector.tensor_scalar_add(
                s0[:, HB + SPL : 2 * HB], pp0[:, HB + SPL : 2 * HB], b0g1
            )

            # products per row-group: R = (side_d + bd) * side_0, fp16
            out_g0 = R3[:, UPB * ub : UPB * (ub + 1), 0:128]
            out_g1 = R3[:, UPB * ub : UPB * (ub + 1), 128:256]
            nc.vector.scalar_tensor_tensor(
                out_g0,
                ppd[:, 0:HB].rearrange("p (u v) -> p u v", v=128),
                bdg0,
                s0[:, 0:HB].rearrange("p (u v) -> p u v", v=128),
                Alu.add,
                Alu.mult,
            )
            nc.vector.scalar_tensor_tensor(
                out_g1,
                ppd[:, HB : 2 * HB].rearrange("p (u v) -> p u v", v=128),
                bdg1,
                s0[:, HB : 2 * HB].rearrange("p (u v) -> p u v", v=128),
                Alu.add,
                Alu.mult,
            )

            # ST staging (gpsimd SWDGE, overlapped): partition-shift the two
            # B-quadrants whose half doesn't match their phase-2 stream.
            u0 = UPB * ub
            if ub < UB // 2:  # u = i < 128: B[i, j<128] from rows 64-127
                nc.gpsimd.dma_start(
                    ST[0:64, 128 * u0 : 128 * (u0 + UPB)],
                    R3[64:128, u0 : u0 + UPB, 0:128],
                )
            else:  # u = i >= 128: B[i, j>=128] from rows 0-63
                nc.gpsimd.dma_start(
                    ST[64:128, 128 * (u0 - 128) : 128 * (u0 - 128 + UPB)],
                    R3[0:64, u0 : u0 + UPB, 128:256],
                )


def _phase2(tc, R3, ST, cout):
    nc = tc.nc
    Copy = mybir.ActivationFunctionType.Copy
    Alu = mybir.AluOpType
    with (
        tc.tile_pool(name="pc", bufs=1, space="PSUM") as pc_pool,
        tc.tile_pool(name="co", bufs=1) as co_pool,
    ):
        pcL = [
            pc_pool.tile([128, 256], F32, tag=f"pcL{t}", name=f"pcL{t}")
            for t in range(2)
        ]
        pcH = [
            pc_pool.tile([128, 256], F32, tag=f"pcH{t}", name=f"pcH{t}")
            for t in range(2)
        ]

        def stream_l(t):
            il = t
            st, sp = (t == 0), (t == 127)
            amov = R3[0:64, :, il]  # A_il^T[f, n] strided [64, 256]
            nc.tensor.matmul(  # j<128: staged stationary
                pcL[0][:], ST[0:64, 128 * il : 128 * (il + 1)], amov,
                start=st, stop=sp, tile_position=(0, 0),
            )
            nc.tensor.matmul(  # j>=128: native stationary
                pcL[1][:], R3[0:64, il, 128:256], amov,
                start=st, stop=sp, tile_position=(0, 0),
            )

        def stream_h(t):
            ih = 128 + t
            st, sp = (t == 0), (t == 127)
            amov = R3[64:128, :, ih]
            nc.tensor.matmul(  # j<128: native stationary
                pcH[0][:], R3[64:128, ih, 0:128], amov,
                start=st, stop=sp, tile_position=(64, 0),
            )
            nc.tensor.matmul(  # j>=128: staged stationary
                pcH[1][:], ST[64:128, 128 * t : 128 * (t + 1)], amov,
                start=st, stop=sp, tile_position=(64, 0),
            )

        if _P2SEQ:
            for t in range(128):
                stream_l(t)
            for t in range(128):
                stream_h(t)
        else:
            for t in range(128):
                stream_l(t)
                stream_h(t)

        # C^T = pcL + pcH; INTERVAL folded into Q0 weights. cout holds C^T.
        c_lo = co_pool.tile([128, 512], F32)
        c_sb = co_pool.tile([128, 512], F32)
        for jt in range(2):
            csl = slice(256 * jt, 256 * (jt + 1))
            nc.scalar.activation(c_lo[:, csl], pcL[jt][:], Copy)
            nc.vector.tensor_tensor(
                c_sb[:, csl], c_lo[:, csl], pcH[jt][:], Alu.add
            )
            nc.sync.dma_start(cout[128 * jt : 128 * (jt + 1), :], c_sb[:, csl])


def _build_nc():
    nc = bacc.Bacc("TRN2", target_bir_lowering=False)
    rbf0 = nc.dram_tensor("rbf0", [X, D], F32, kind="ExternalInput")
    rbfd = nc.dram_tensor("rbfd", [X, D], F32, kind="ExternalInput")
    wpack = nc.dram_tensor("wpack", [128, 256], BF16, kind="ExternalInput")
    bpack = nc.dram_tensor("bpack", [128, 4], F32, kind="ExternalInput")
    cout = nc.dram_tensor("c", [N, N], F32, kind="ExternalOutput")
    with tile.TileContext(nc) as tc:
        with ExitStack() as ctx:
            _body(ctx, tc, rbf0, rbfd, wpack, bpack, cout)
    nc.compile()
    return nc


_CACHE = {}


def _get_nc():
    if "nc" not in _CACHE:
        _CACHE["nc"] = _build_nc()
    return _CACHE["nc"]


def _make_in_maps(inp):
    # pre-round inputs to bf16 so the on-chip bf16 truncation is lossless
    def r16(x):
        x = np.asarray(x, dtype=np.float32)
        return np.ascontiguousarray(
            x.astype(ml_dtypes.bfloat16).astype(np.float32)
        )

    rbf_0 = r16(inp["rbf_0"])
    rbf_d = r16(inp["rbf_d"])

    # INTERVAL folded into the Q0 family (scales A, hence C)
    q0w = np.asarray(inp["Q0_w"], dtype=np.float64) * INTERVAL
    q0b = np.asarray(inp["Q0_b"], dtype=np.float64) * INTERVAL

    # stationary stacking: rows 0-63 (g0, v<128) = [Q|R] columns,
    # rows 64-127 (g1, v>=128) = [R|Q] (swapped). wpack = [w0 | wd].
    def wstack(wq, wr):
        wq, wr = np.asarray(wq, np.float64), np.asarray(wr, np.float64)
        g0 = np.concatenate([wq.T, wr.T], axis=1)  # [64, 128]
        g1 = np.concatenate([wr.T, wq.T], axis=1)
        return np.concatenate([g0, g1], axis=0)  # [128, 128]

    w0 = wstack(q0w, inp["R0_w"])
    wd = wstack(inp["Q_w"], inp["R_w"])
    wpack = np.concatenate([w0, wd], axis=1).astype(ml_dtypes.bfloat16)

    def bstack(bq, br):
        bq, br = np.asarray(bq, np.float64), np.asarray(br, np.float64)
        g0 = np.concatenate([bq, br])
        g1 = np.concatenate([br, bq])
        return g0, g1

    b0g0, b0g1 = bstack(q0b, inp["R0_b"])
    bdg0, bdg1 = bstack(inp["Q_b"], inp["R_b"])
    bpack = np.stack([b0g0, b0g1, bdg0, bdg1], axis=1).astype(np.float32)

    return [
        {
            "rbf0": rbf_0[b].reshape(X, D),
            "rbfd": rbf_d[b].reshape(X, D),
            "wpack": wpack,
            "bpack": bpack,
        }
        for b in range(B)
    ]


def kernel(**inputs):
    in_maps = _make_in_maps(inputs)
    nc = _get_nc()
    res = run_bass_kernel_spmd(nc, in_maps, core_ids=list(range(B)))
    # cout holds C^T; transpose back on the host
    return np.stack(
        [np.ascontiguousarray(res.results[b]["c"].T) for b in range(B)], axis=0
    )


if __name__ == "__main__":
    import reference

    inp = {k: np.asarray(v) for k, v in reference.setup_inputs().items()}
    got = kernel(**inp)
    exp = np.asarray(reference.reference(**inp))
    err = np.abs(got - exp)
    print("absmax_err", err.max(), "rel", err.max() / np.abs(exp).max())


# revision 8
# speedup vs baseline: 1.1658x; 1.1658x over previous
"""Trainium2 Bass kernel: CorrelatorK3.

Math (per batch b):
    q0 = rbf_0 @ Q0_w.T + Q0_b          [N, N, F]
    q  = rbf_d @ Q_w.T  + Q_b
    r0 = rbf_0 @ R0_w.T + R0_b
    r  = rbf_d @ R_w.T  + R_b
    C[n, j] = sum_{i, f} (q0*q)[n, i, f] * (r0*r)[i, j, f] * 0.02

Sharding: data-parallel over batch B=8 across the 8 NeuronCores (one batch
per core); output is a pure concat. INTERVAL=0.02 is folded into the Q0
weights/bias on the host; inputs are pre-rounded to bf16 on the host so the
on-chip bf16 truncation is lossless.

Per-core pipeline:
  Phase 1 (stream x = flattened (u, v) index over 256*256 rows):
    - DMA natural [x, d] fp32 tiles (rbf0 on sync HWDGE, rbfd on gpsimd
      SWDGE). PE-transposes operate on the bf16 upper halves of the fp32
      words (bitcast, stride 2) with a bf16 identity -> bf16 PSUM out at
      1 cyc/col; ScalarE evacuates psum->SBUF bf16 (2x mode).
    - Projections as bf16 matmuls. Stationary rows 0-63 serve the v<128
      column half (g0) with [Q|R]-stacked weight columns; rows 64-127
      serve v>=128 (g1) with the stacking SWAPPED to [R|Q]; the two
      row-groups run concurrently via tile_position.
    - Bias + products fused with PSUM evacuation (ScalarE + VectorE,
      split for engine balance), fp16 into resident R (x-layout):
        v<128:  R[0:64]  = A^T[f,(n,i)]   R[64:128] = B^T[f,(i,j)]
        v>=128: R[0:64]  = B^T[f,(i,j)]   R[64:128] = A^T[f,(n,i)]
      where A=(0.02(q0+b))(q+b) at x=(n,i), B=(r0+b)(r+b) at x=(i,j).
    - ST staging (gpsimd queue, overlapped per u-block): partition-shift
      the two B-quadrants whose half doesn't match their phase-2 stream:
        ST[0:64,  i*128+j] = B[i<128,  j<128]   (from R rows 64-127)
        ST[64:128,(i-128)*128+(j-128)] = B[i>=128, j>=128] (from rows 0-63)
  Phase 2 (contraction as C^T, two PE row-tile streams):
    C^T[j, n] = sum_i B_i^T[f, j]^T A_i^T[f, n]
    - stream L (rows 0-63):  i<128, stationary = contiguous B_i slices
      (staged ST for j<128, native R for j>=128), moving = strided
      A_i^T[f, :] slice; accumulate pcL0/pcL1.
    - stream H (rows 64-127): i>=128 symmetric; pcH0/pcH1.
    - C^T = pcL + pcH (ScalarE copy + VectorE add), DMA out; the host
      transposes back to C.
"""

import os
import sys

if "/opt/trn_rl_repo" not in sys.path:
    sys.path.insert(0, "/opt/trn_rl_repo")

from contextlib import ExitStack

import ml_dtypes
import numpy as np

import concourse.mybir as mybir
import concourse.tile as tile
from concourse import bacc
from concourse.bass_utils import run_bass_kernel_spmd
from concourse.masks import make_identity

B, N, D, F = 8, 256, 64, 64
X = N * N  # 65536 flattened rows per batch
INTERVAL = 0.02

F32 = mybir.dt.float32
F16 = mybir.dt.float16
BF16 = mybir.dt.bfloat16

UPB = int(os.environ.get("KERNEL_UPB", "4"))  # u-rows per phase-1 block
UB = X // (UPB * N)  # 64 phase-1 blocks
_PHASES = os.environ.get("KERNEL_PHASES", "12")  # debug: "1" or "2" only
_P2SEQ = os.environ.get("KERNEL_P2SEQ", "0") == "1"  # sequential streams
_P2FORM = os.environ.get("KERNEL_P2FORM", "c")  # "c" or "ct" (fallback)
SPL = 384  # s0-g1 column split: [0:SPL] on ScalarE, [SPL:] on VectorE


def _body(ctx, tc, rbf0, rbfd, wpack, bpack, cout):
    nc = tc.nc

    const = ctx.enter_context(tc.tile_pool(name="const", bufs=1))
    w_sb = const.tile([128, 256], BF16)
    b_sb = const.tile([128, 4], F32)
    ident = const.tile([128, 128], BF16)
    nc.sync.dma_start(w_sb[:], wpack[:])
    nc.sync.dma_start(b_sb[:], bpack[:])
    make_identity(nc, ident[:])

    res_pool = ctx.enter_context(tc.tile_pool(name="res", bufs=1))
    R = res_pool.tile([128, X], F16)
    R3 = R[:].rearrange("p (u v) -> p u v", v=N)  # [128, 256, 256]
    ST = res_pool.tile([128, 128 * 128], F16)  # staged B quadrants

    rbf0v = rbf0[:].rearrange("(c t p) d -> c p t d", t=2 * UPB, p=128)
    rbfdv = rbfd[:].rearrange("(c t p) d -> c p t d", t=2 * UPB, p=128)

    if "1" in _PHASES:
        _phase1(tc, rbf0v, rbfdv, w_sb, b_sb, ident, R3, ST)
    if "2" in _PHASES:
        _phase2(tc, R3, ST, cout)


def _phase1(tc, rbf0v, rbfdv, w_sb, b_sb, ident, R3, ST):
    nc = tc.nc
    Copy = mybir.ActivationFunctionType.Copy
    Ident = mybir.ActivationFunctionType.Identity
    Alu = mybir.AluOpType
    HB = UPB * 128  # half-block columns (one row-group's share)
    b0g0, b0g1 = b_sb[:, 0:1], b_sb[:, 1:2]
    bdg0, bdg1 = b_sb[:, 2:3], b_sb[:, 3:4]
    with (
        tc.tile_pool(name="chunk", bufs=3) as chunk_pool,
        tc.tile_pool(name="rbfT", bufs=2) as rbfT_pool,
        tc.tile_pool(name="s0p", bufs=2) as s0_pool,
        tc.tile_pool(name="pt", bufs=2, space="PSUM") as pt_pool,
        tc.tile_pool(
            name="pp", bufs=(2 if UPB <= 2 else 1), space="PSUM"
        ) as pp_pool,
    ):
        for ub in range(UB):
            ch0 = chunk_pool.tile([128, HB], F32, tag="ch0")
            chd = chunk_pool.tile([128, HB], F32, tag="chd")
            nc.sync.dma_start(
                ch0[:].rearrange("p (t d) -> p t d", d=D), rbf0v[ub]
            )
            nc.gpsimd.dma_start(
                chd[:].rearrange("p (t d) -> p t d", d=D), rbfdv[ub]
            )

            # bf16 view of the fp32 words: [p, col, lo/hi]; hi = bf16 trunc
            c0h = ch0[:].bitcast(BF16).rearrange("p (c two) -> p c two", two=2)
            cdh = chd[:].bitcast(BF16).rearrange("p (c two) -> p c two", two=2)

            # transpose UPB [128, 128] bf16 sub-blocks per side into one
            # psum tile (each transpose stays inside one bank)
            pt = pt_pool.tile([128, 2 * HB], BF16, tag="pt")
            for j in range(UPB):
                sl = slice(128 * j, 128 * (j + 1))
                sld = slice(HB + 128 * j, HB + 128 * (j + 1))
                nc.tensor.transpose(pt[:, sl], c0h[:, sl, 1], ident[:])
                nc.tensor.transpose(pt[:, sld], cdh[:, sl, 1], ident[:])

            # evacuate both sides' transposes (bf16 2x mode on ScalarE)
            tt = rbfT_pool.tile([128, 2 * HB], BF16, tag="tt")
            nc.scalar.activation(tt[:], pt[:], Copy)

            # projections: wide psum per side, col-halves per row-group.
            # g0 (v<128) stationary rows 0-63: [Q|R]; g1 rows 64-127: [R|Q]
            pp0 = pp_pool.tile([128, 2 * HB], F32, tag="pp0")
            ppd = pp_pool.tile([128, 2 * HB], F32, tag="ppd")
            nc.tensor.matmul(
                pp0[:, 0:HB], w_sb[0:64, 0:128], tt[0:64, 0:HB],
                start=True, stop=True, tile_position=(0, 0),
            )
            nc.tensor.matmul(
                ppd[:, 0:HB], w_sb[0:64, 128:256], tt[0:64, HB : 2 * HB],
                start=True, stop=True, tile_position=(0, 0),
            )
            nc.tensor.matmul(
                pp0[:, HB : 2 * HB], w_sb[64:128, 0:128], tt[64:128, 0:HB],
                start=True, stop=True, tile_position=(64, 0),
            )
            nc.tensor.matmul(
                ppd[:, HB : 2 * HB], w_sb[64:128, 128:256],
                tt[64:128, HB : 2 * HB],
                start=True, stop=True, tile_position=(64, 0),
            )

            # side-0 bias evacuation, split for engine balance
            s0 = s0_pool.tile([128, 2 * HB], F32, tag="s0")
            nc.scalar.activation(
                s0[:, 0:HB], pp0[:, 0:HB], Ident, bias=b0g0
            )
            nc.scalar.activation(
                s0[:, HB : HB + SPL], pp0[:, HB : HB + SPL], Ident, bias=b0g1
            )
            nc.vector.tensor_scalar_add(
                s0[:, HB + SPL : 2 * HB], pp0[:, HB + SPL : 2 * HB], b0g1
            )

            # products per row-group: R = (side_d + bd) * side_0, fp16
            out_g0 = R3[:, UPB * ub : UPB * (ub + 1), 0:128]
            out_g1 = R3[:, UPB * ub : UPB * (ub + 1), 128:256]
            nc.vector.scalar_tensor_tensor(
                out_g0,
                ppd[:, 0:HB].rearrange("p (u v) -> p u v", v=128),
                bdg0,
                s0[:, 0:HB].rearrange("p (u v) -> p u v", v=128),
                Alu.add,
                Alu.mult,
            )
            nc.vector.scalar_tensor_tensor(
                out_g1,
                ppd[:, HB : 2 * HB].rearrange("p (u v) -> p u v", v=128),
                bdg1,
                s0[:, HB : 2 * HB].rearrange("p (u v) -> p u v", v=128),
                Alu.add,
                Alu.mult,
            )

            # ST staging (sync HWDGE, overlapped): partition-shift the two
            # B-quadrants whose half doesn't match their phase-2 stream.
            u0 = UPB * ub
            if ub < UB // 2:  # u = i < 128: B[i, j<128] from rows 64-127
                nc.sync.dma_start(
                    ST[0:64, 128 * u0 : 128 * (u0 + UPB)],
                    R3[64:128, u0 : u0 + UPB, 0:128],
                )
            else:  # u = i >= 128: B[i, j>=128] from rows 0-63
                nc.sync.dma_start(
                    ST[64:128, 128 * (u0 - 128) : 128 * (u0 - 128 + UPB)],
                    R3[0:64, u0 : u0 + UPB, 128:256],
                )


def _phase2(tc, R3, ST, cout):
    nc = tc.nc
    Copy = mybir.ActivationFunctionType.Copy
    Alu = mybir.AluOpType
    with (
        tc.tile_pool(name="pc", bufs=1, space="PSUM") as pc_pool,
        tc.tile_pool(name="co", bufs=1) as co_pool,
    ):
        pcL = [
            pc_pool.tile([128, 256], F32, tag=f"pcL{t}", name=f"pcL{t}")
            for t in range(2)
        ]
        pcH = [
            pc_pool.tile([128, 256], F32, tag=f"pcH{t}", name=f"pcH{t}")
            for t in range(2)
        ]

        if _P2FORM == "c":
            # C-form: stationary = strided A_i^T[f, n-tile] (LDW-bound but
            # pipelined); moving = contiguous 128-col B_i j-halves. Both
            # MMs of one stationary are adjacent. pcX[nt] = C[n-tile] part.
            def stream_l(t, nt):
                il = t
                st, sp = (t == 0), (t == 127)
                nsl = slice(128 * nt, 128 * (nt + 1))
                stat = R3[0:64, nsl, il : il + 1]
                nc.tensor.matmul(
                    pcL[nt][:, 0:128], stat,
                    ST[0:64, 128 * il : 128 * (il + 1)],
                    start=st, stop=sp, tile_position=(0, 0),
                )
                nc.tensor.matmul(
                    pcL[nt][:, 128:256], stat, R3[0:64, il, 128:256],
                    start=st, stop=sp, tile_position=(0, 0),
                )

            def stream_h(t, nt):
                ih = 128 + t
                st, sp = (t == 0), (t == 127)
                nsl = slice(128 * nt, 128 * (nt + 1))
                stat = R3[64:128, nsl, ih : ih + 1]
                nc.tensor.matmul(
                    pcH[nt][:, 0:128], stat, R3[64:128, ih, 0:128],
                    start=st, stop=sp, tile_position=(64, 0),
                )
                nc.tensor.matmul(
                    pcH[nt][:, 128:256], stat,
                    ST[64:128, 128 * t : 128 * (t + 1)],
                    start=st, stop=sp, tile_position=(64, 0),
                )

            if _P2SEQ:
                for t in range(128):
                    for nt in range(2):
                        stream_l(t, nt)
                for t in range(128):
                    for nt in range(2):
                        stream_h(t, nt)
            else:
                for t in range(128):
                    stream_l(t, 0)
                    stream_h(t, 0)
                    stream_l(t, 1)
                    stream_h(t, 1)
            # C[n-tile] = pcL[nt] + pcH[nt]
            c_lo = co_pool.tile([128, 512], F32)
            c_sb = co_pool.tile([128, 512], F32)
            for nt in range(2):
                csl = slice(256 * nt, 256 * (nt + 1))
                nc.scalar.activation(c_lo[:, csl], pcL[nt][:], Copy)
                nc.vector.tensor_tensor(
                    c_sb[:, csl], c_lo[:, csl], pcH[nt][:], Alu.add
                )
                nc.sync.dma_start(
                    cout[128 * nt : 128 * (nt + 1), :], c_sb[:, csl]
                )
            return

        # C^T-form (fallback): stationary = contiguous B_i slices,
        # moving = strided A_i^T[f, :]. cout holds C^T (host transposes).
        def ct_stream_l(t):
            il = t
            st, sp = (t == 0), (t == 127)
            amov = R3[0:64, :, il]
            nc.tensor.matmul(
                pcL[0][:], ST[0:64, 128 * il : 128 * (il + 1)], amov,
                start=st, stop=sp, tile_position=(0, 0),
            )
            nc.tensor.matmul(
                pcL[1][:], R3[0:64, il, 128:256], amov,
                start=st, stop=sp, tile_position=(0, 0),
            )

        def ct_stream_h(t):
            ih = 128 + t
            st, sp = (t == 0), (t == 127)
            amov = R3[64:128, :, ih]
            nc.tensor.matmul(
                pcH[0][:], R3[64:128, ih, 0:128], amov,
                start=st, stop=sp, tile_position=(64, 0),
            )
            nc.tensor.matmul(
                pcH[1][:], ST[64:128, 128 * t : 128 * (t + 1)], amov,
                start=st, stop=sp, tile_position=(64, 0),
            )

        if _P2SEQ:
            for t in range(128):
                ct_stream_l(t)
            for t in range(128):
                ct_stream_h(t)
        else:
            for t in range(128):
                ct_stream_l(t)
                ct_stream_h(t)

        c_lo = co_pool.tile([128, 512], F32)
        c_sb = co_pool.tile([128, 512], F32)
        for jt in range(2):
            csl = slice(256 * jt, 256 * (jt + 1))
            nc.scalar.activation(c_lo[:, csl], pcL[jt][:], Copy)
            nc.vector.tensor_tensor(
                c_sb[:, csl], c_lo[:, csl], pcH[jt][:], Alu.add
            )
            nc.sync.dma_start(cout[128 * jt : 128 * (jt + 1), :], c_sb[:, csl])


def _build_nc():
    nc = bacc.Bacc("TRN2", target_bir_lowering=False)
    rbf0 = nc.dram_tensor("rbf0", [X, D], F32, kind="ExternalInput")
    rbfd = nc.dram_tensor("rbfd", [X, D], F32, kind="ExternalInput")
    wpack = nc.dram_tensor("wpack", [128, 256], BF16, kind="ExternalInput")
    bpack = nc.dram_tensor("bpack", [128, 4], F32, kind="ExternalInput")
    cout = nc.dram_tensor("c", [N, N], F32, kind="ExternalOutput")
    with tile.TileContext(nc) as tc:
        with ExitStack() as ctx:
            _body(ctx, tc, rbf0, rbfd, wpack, bpack, cout)
    nc.compile()
    return nc


_CACHE = {}


def _get_nc():
    if "nc" not in _CACHE:
        _CACHE["nc"] = _build_nc()
    return _CACHE["nc"]


def _make_in_maps(inp):
    # pre-round inputs to bf16 so the on-chip bf16 truncation is lossless
    def r16(x):
        x = np.asarray(x, dtype=np.float32)
        return np.ascontiguousarray(
            x.astype(ml_dtypes.bfloat16).astype(np.float32)
        )

    rbf_0 = r16(inp["rbf_0"])
    rbf_d = r16(inp["rbf_d"])

    # INTERVAL folded into the Q0 family (scales A, hence C)
    q0w = np.asarray(inp["Q0_w"], dtype=np.float64) * INTERVAL
    q0b = np.asarray(inp["Q0_b"], dtype=np.float64) * INTERVAL

    # stationary stacking: rows 0-63 (g0, v<128) = [Q|R] columns,
    # rows 64-127 (g1, v>=128) = [R|Q] (swapped). wpack = [w0 | wd].
    def wstack(wq, wr):
        wq, wr = np.asarray(wq, np.float64), np.asarray(wr, np.float64)
        g0 = np.concatenate([wq.T, wr.T], axis=1)  # [64, 128]
        g1 = np.concatenate([wr.T, wq.T], axis=1)
        return np.concatenate([g0, g1], axis=0)  # [128, 128]

    w0 = wstack(q0w, inp["R0_w"])
    wd = wstack(inp["Q_w"], inp["R_w"])
    wpack = np.concatenate([w0, wd], axis=1).astype(ml_dtypes.bfloat16)

    def bstack(bq, br):
        bq, br = np.asarray(bq, np.float64), np.asarray(br, np.float64)
        g0 = np.concatenate([bq, br])
        g1 = np.concatenate([br, bq])
        return g0, g1

    b0g0, b0g1 = bstack(q0b, inp["R0_b"])
    bdg0, bdg1 = bstack(inp["Q_b"], inp["R_b"])
    bpack = np.stack([b0g0, b0g1, bdg0, bdg1], axis=1).astype(np.float32)

    return [
        {
            "rbf0": rbf_0[b].reshape(X, D),
            "rbfd": rbf_d[b].reshape(X, D),
            "wpack": wpack,
            "bpack": bpack,
        }
        for b in range(B)
    ]


def kernel(**inputs):
    in_maps = _make_in_maps(inputs)
    nc = _get_nc()
    res = run_bass_kernel_spmd(nc, in_maps, core_ids=list(range(B)))
    if _P2FORM == "c":
        return np.stack([res.results[b]["c"] for b in range(B)], axis=0)
    # C^T-form: transpose back on the host
    return np.stack(
        [np.ascontiguousarray(res.results[b]["c"].T) for b in range(B)], axis=0
    )


if __name__ == "__main__":
    import reference

    inp = {k: np.asarray(v) for k, v in reference.setup_inputs().items()}
    got = kernel(**inp)
    exp = np.asarray(reference.reference(**inp))
    err = np.abs(got - exp)
    print("absmax_err", err.max(), "rel", err.max() / np.abs(exp).max())
